# revision 2
# baseline (speedup 1.0000x reference)
"""CompensatedSparseLinear on 8 TRN2 NeuronCores.

out[b,s,o] = sum_i x[b,s,i] * (W[o,i] + delta[o,i]) + b[o]

The sparse COO delta is folded into W on the host (scatter-add), leaving a
dense matmul: out2d = x2d @ W_eff^T + b with x2d [8192, 4096], W_eff [4096, 4096].

Sharding: data-parallel along the 8192 batch*seq rows — 1024 rows per core,
W_eff/b replicated. No collectives; host concatenates the output shards.

Per-core device kernel (out^T layout — out_features on PSUM partitions):
  outT[nt*128+ni, m] = sum_k W_eff[nt*128+ni, k] * x[m, k] + b[nt*128+ni]
  - operands in bfloat16 (fp32 PSUM accumulate): same 1 cyc/row PE rate as
    float32r but half the DMA traffic and SBUF footprint; rel absmax err
    ~1.6e-3 vs the 2e-2 gate (measured; 1.1e-4 for fp32r)
  - x^T shard resident in SBUF as 32 per-k tiles [ki, m] (64 KB/partition
    total): per-chunk DMA dependency granularity, so the first matmul
    waits only on its own k-chunk (~5 us into the cold run), not on the
    whole 8 MB x stream — matters for the graded single-shot
  - W_eff^T streamed per n-tile as [ki, (k, ni)] blocks (8 KB/partition,
    4-deep buffered), 4 DMAs per n-tile from contiguous host-pretiled DRAM
  - bias added via ScalarE activation(Identity, bias) — per-partition bias
  - post-compile BIR pass (_dedupe_ldweights) removes the redundant second
    LDWEIGHTS of each same-weight matmul pair: the tile legalizer
    (maybe_split_ldweights) splits EVERY InstMatmult into a standalone
    LDWEIGHTS + non-self-loading MATMUL, so back-to-back matmuls on the
    same k-slice reload identical weights. Dedupe halves the weight-load
    count (2048 -> 1024); output verified bit-identical.

Optimization notes (2026-08-10 session — read before optimizing further):
- THE KERNEL IS AT THE DEVICE'S POWER-LIMITED FLOOR. The ~100ns/MM gap
  over the 437us@2.4GHz PE floor that the 2026-08-08 session attributed
  to per-matmul instruction overhead is CLOCK THROTTLING: under
  sustained full-array bf16 load the PE runs at ~2.0-2.1 GHz effective
  (power cap / P0 state), not 2.4. Cool-burst protocol (R2=504 with 25s
  cooldowns, latin-square order): EVERY structural variant measures
  507-526 us/iter (= 2048 MMs x 512cyc @ ~2.0GHz). Saturated sustained
  protocol (R2=2004 back-to-back): 620-860 us/iter, monotonic drift with
  device heat. Kernel-structure deltas are <=8 us in both regimes.
- Evidence for the power model: a microbench using only 32 of 128 PE
  array columns (1/4 power) runs at 409-457us with everything else
  identical; LDWEIGHTS count 2048 vs 1 (same-weight loads, deduped to a
  single load) measures IDENTICAL (508 vs 511us); removing 99% of the
  32MB W DMA saves only 8us; PSUM bufs 4 vs 8 nil; TP2xDP4 resharding
  (4 MMs per weight load, 512 loads, 32MB less DMA) is +8us WORSE cool.
  LDWEIGHTS are fully hidden by hardware in all regimes measured.
- Things that DON'T work (tried 2026-08-10):
  - InstMatmult.ldweights flag: serialized to BIR but walrus ignores it.
  - walrus --enable-ldw-opt=true (run_command argv patch): rejects
    standalone InstLdweights ("not compatible with LDW optimization");
    after deleting them (matmuls still carry weights operands) it
    compiles to ZERO LDWEIGHTS in the NEFF and returns NaN — the weight
    loads are dropped entirely, not folded. Unusable.
  - Explicit nc.tensor.ldweights + matmuls: scheduler floats the
    standalone loads away from their matmuls; walrus still emits the
    per-matmul pair -> 3072 loads. Strictly worse.
  - fp8 (2-pass hi/lo): DoubleRow's ~1.44x rate x2 passes loses to bf16
    1-pass; 1-pass precision ~3.6% >> 2e-2 gate. Dead on arrival.
  - MB=1024 (2 PSUM banks per matmul) rejected by walrus codegen.
- Measurement discipline: the shared device's thermal state dominates
  kernel deltas. Use bench_ab2.py (latin-square, 25s cooldowns, R2=504)
  to resolve <5% differences; treat any sequential-process comparison as
  +-10%. r1 walls of 160-180ms (vs ~105ms) are dispatch hiccups — drop
  those rounds.
- Remaining theoretical levers, all blocked: fewer PE cycles needs fp8
  (precision-dead); higher clock needs less power/cycle (operand
  activity is data-random); N>512 per matmul needs multi-bank PSUM
  writes (hardware limit).

Measured (in-NEFF For_i repeat differencing, R=4 vs 2004, min-of-5,
device-resident inputs, same protocol as the 666393ns baseline number):
see test.py. Cool-burst per-iter is ~517us; saturated sustained ~650-720.
"""

import numpy as np
import ml_dtypes

import concourse.bacc as bacc
import concourse.tile as tile
import concourse.mybir as mybir
from concourse.bass_utils import run_bass_kernel_spmd

# Problem shape (hardcoded — harness contract)
B, S, D_IN, D_OUT = 4, 2048, 4096, 4096
N_CORES = 8
M_TOT = B * S              # 8192 rows
M = M_TOT // N_CORES       # 1024 rows per core
KT = D_IN // 128           # 32 k tiles
NT = D_OUT // 128          # 32 n tiles
MB = 512                   # moving free-dim per matmul (PSUM bank)
NMB = M // MB              # 2 m-blocks per core
WCH = 4                    # DMA chunks per n-tile W block (alternating SP/Act queues)
W_BUFS = 4
O_BUFS = 3

_MM_DT = mybir.dt.bfloat16
_NP_MM = ml_dtypes.bfloat16

_CACHE: dict = {}
LAST = {"exec_time_ns": None}


def _ldw_sig(i):
    ap = i.ins[0]
    return (ap.memref, ap.offset, str(ap.ap), str(ap.dtype))


def _dedupe_ldweights(nc):
    """Remove consecutive duplicate InstLdweights (same weights AP, no
    sync_info, only matmuls/other-engine instrs between) from each block.

    The tile legalizer splits every InstMatmult into LDWEIGHTS+MATMUL, so
    the second matmul of each same-weight m-block pair carries a redundant
    reload of identical weights. Removing it halves the weight-load count;
    output is bit-identical (verified vs the jax reference)."""
    n_removed = 0
    for f in nc.m.functions:
        for b in f.blocks:
            out = []
            last_sig = None
            for i in b.instructions:
                if isinstance(i, mybir.InstLdweights):
                    sig = _ldw_sig(i)
                    si = i.sync_info
                    bare = si is None or (
                        len(si.on_wait) == 0 and len(si.on_update) == 0)
                    if bare and sig == last_sig:
                        n_removed += 1
                        continue
                    last_sig = sig
                elif isinstance(i, mybir.InstMatmult):
                    pass
                elif i.engine == mybir.EngineType.PE:
                    last_sig = None
                out.append(i)
            b.instructions[:] = out
    return n_removed


def _build(repeat=1):
    nc = bacc.Bacc("TRN2", target_bir_lowering=False, debug=False)

    # xT[ki, k, m] = x_shard[m, k*128+ki]
    xT = nc.declare_dram_parameter("xT", [128, KT, M], _MM_DT, isOutput=False)
    # wt[nt, ki, k, ni] = W_eff[nt*128+ni, k*128+ki]
    wt = nc.declare_dram_parameter("wt", [NT, 128, KT, 128], _MM_DT, isOutput=False)
    # bias[ni, nt] = b[nt*128+ni]
    bias = nc.declare_dram_parameter("bias", [128, NT], mybir.dt.float32, isOutput=False)
    # outT[nt, ni, m]
    outT = nc.declare_dram_parameter("outT", [NT, 128, M], mybir.dt.float32, isOutput=True)

    with tile.TileContext(nc) as tc:
        with (
            tc.tile_pool(name="xp", bufs=KT) as xp,
            tc.tile_pool(name="bp", bufs=1) as bp,
            tc.tile_pool(name="wp", bufs=W_BUFS) as wp,
            tc.tile_pool(name="ps", bufs=8, space="PSUM") as ps,
            tc.tile_pool(name="op", bufs=O_BUFS) as op,
        ):
            def body(_iv=None):
                kc = KT // WCH  # k-tiles per W DMA chunk

                def load_w(nt):
                    # W block for this n-tile: [ki, (k, ni)] — 8 KB/partition.
                    # Chunks alternate between the two HWDGE trigger queues
                    # (qSPDynamicHW / qActDynamicHW) to double DMA issue BW.
                    w = wp.tile([128, KT * 128], _MM_DT, tag="w", name=f"w_{nt}")
                    for j in range(WCH):
                        eng = nc.sync if j % 2 == 0 else nc.scalar
                        eng.dma_start(
                            w[:, j * kc * 128 : (j + 1) * kc * 128],
                            wt[nt, :, j * kc : (j + 1) * kc, :],
                        )
                    return w

                bs = bp.tile([128, NT], mybir.dt.float32, name="bs")
                nc.sync.dma_start(bs[:], bias[:])

                # Cold-start critical path: the first matmul needs w0 and
                # xs[0] only. Issue w0 and the first x chunks before w1 so
                # the PE starts ~2 us earlier on the graded single shot.
                w_fifo = [load_w(0)]

                # resident x^T shard as KT per-k tiles (2 KB/partition each):
                # dependency granularity = one DMA chunk, not the whole shard
                xs = []

                def load_x(k):
                    xk = xp.tile([128, M], _MM_DT, tag="xs", name=f"xs_{k}")
                    eng = nc.sync if k % 2 == 0 else nc.scalar
                    eng.dma_start(xk[:], xT[:, k, :])
                    xs.append(xk)

                for k in range(4):
                    load_x(k)
                w_fifo.append(load_w(1))
                for k in range(4, KT):
                    load_x(k)

                for nt in range(NT):
                    # keep W prefetch 2 n-tiles ahead of the PE
                    w = w_fifo.pop(0)
                    if nt + 2 < NT:
                        w_fifo.append(load_w(nt + 2))
                    accs = [
                        ps.tile([128, MB], mybir.dt.float32, tag="ps", name=f"acc_{nt}_{mb}")
                        for mb in range(NMB)
                    ]
                    for k in range(KT):
                        for mb in range(NMB):
                            nc.tensor.matmul(
                                accs[mb][:],
                                w[:, k * 128 : (k + 1) * 128],
                                xs[k][:, mb * MB : (mb + 1) * MB],
                                start=(k == 0),
                                stop=(k == KT - 1),
                            )
                    o = op.tile([128, M], mybir.dt.float32, tag="o", name=f"o_{nt}")
                    for mb in range(NMB):
                        nc.scalar.activation(
                            o[:, mb * MB : (mb + 1) * MB],
                            accs[mb][:],
                            mybir.ActivationFunctionType.Identity,
                            bias=bs[:, nt : nt + 1],
                        )
                    # out DMA via gpsimd SWDGE: keeps the SP/Act HWDGE queues free for W prefetch
                    nc.gpsimd.dma_start(outT[nt], o[:])

            if repeat == 1:
                body()
            else:
                with tc.For_i(0, repeat, 1) as _i:
                    body(_i)

    nc.compile()
    _dedupe_ldweights(nc)
    return nc


def make_in_maps(x, W, b, delta_vals, delta_rows, delta_cols):
    x = np.asarray(x, dtype=np.float32)
    W = np.asarray(W, dtype=np.float32)
    b = np.asarray(b, dtype=np.float32)

    # Fold sparse delta into W (duplicate coords sum)
    W_eff = W.copy()
    np.add.at(
        W_eff,
        (np.asarray(delta_rows), np.asarray(delta_cols)),
        np.asarray(delta_vals, dtype=np.float32),
    )

    # wt[nt, ki, k, ni] = W_eff[nt*128+ni, k*128+ki]
    w_tiles = np.ascontiguousarray(
        W_eff.reshape(NT, 128, KT, 128).transpose(0, 3, 2, 1)
    ).astype(_NP_MM)
    bias_t = np.ascontiguousarray(b.reshape(NT, 128).T)

    x2d = x.reshape(M_TOT, D_IN)
    in_maps = []
    for c in range(N_CORES):
        shard = x2d[c * M : (c + 1) * M]  # [m, (k ki)]
        xT_c = np.ascontiguousarray(
            shard.reshape(M, KT, 128).transpose(2, 1, 0)
        ).astype(_NP_MM)
        in_maps.append({"xT": xT_c, "wt": w_tiles, "bias": bias_t})
    return in_maps


def kernel(x, W, b, delta_vals, delta_rows, delta_cols):
    in_maps = make_in_maps(x, W, b, delta_vals, delta_rows, delta_cols)

    if "nc" not in _CACHE:
        _CACHE["nc"] = _build()
    nc = _CACHE["nc"]

    res = run_bass_kernel_spmd(nc, in_maps, list(range(N_CORES)))
    LAST["exec_time_ns"] = res.exec_time_ns

    out2d = np.empty((M_TOT, D_OUT), dtype=np.float32)
    for c in range(N_CORES):
        outT_c = res.results[c]["outT"].reshape(D_OUT, M)  # [4096, 1024]
        out2d[c * M : (c + 1) * M] = outT_c.T
    return out2d.reshape(B, S, D_OUT)
